# revision 32
# baseline (speedup 1.0000x reference)
"""Distributed attention kernel for 8 TRN2 NeuronCores (natural-layout pipeline).

Problem: x[8192,1024] @ {W_q,W_k,W_v}[1024,128] -> softmax(QK^T/sqrt(128)) @ V.

Sharding: x row-sharded (1024 rows/core), weights replicated. Each core
computes K^T_loc/V_loc from its shard, AllGathers K^T (f32r) and V (bf16),
then attends its own 1024 Q rows against the full K/V.

Per-core attention (natural layout: q on partitions), per 128-row q-tile,
software-pipelined across tiles so phase (a) of tile t+1 overlaps (b) of t:
  (a) S pass: for each 512-kv chunk, S = Q_tile.T-stationary @ K^T-moving
      (f32r, one matmul); ONE fused DVE op (tensor_tensor_reduce) copies the
      PSUM chunk to SBUF *and* folds its row-max into a running max. S is
      computed exactly once (no separate stats matmul pass).
  (b) exp/AV pass: negm = -(m_hat)-1; for each chunk:
        A = exp(S_sbuf + negm)  [ACT: per-partition bias = per-q shift; the
            same instruction's accum_out emits the per-q denominator partial
            -- den costs no matmuls at all]
        A^T blocks via PE transpose (bf16) -> PSUM -> SBUF copy (split
            between DVE and ACT to balance engine load)
        O[q,dv] += A^T_block-stationary @ V_tile-moving  [bf16]
      Then den = sum of partials, O *= 1/den (DVE), direct DMA out (output
      is already in natural row layout -- no final transposes).

Numerics: logits have std ~1024 (randn inputs); softmax is near-one-hot so
the Q/K/S path needs |logit err| << 1: f32r (11-bit mantissa) gives ~0.15.
m_hat is the exact f32 row max of the f32r logits; the -1 bias is cosmetic.
"""

import os
import sys

import numpy as np

os.environ.setdefault("MYCRO_LOCAL_CACHE", "1")

try:
    import concourse  # noqa: F401
except ImportError:  # pragma: no cover - path fallback for fresh dirs
    for _p in ("/opt/trn_rl_repo", "/root/.axon_site/_ro/trn_rl_repo"):
        if os.path.isdir(_p):
            sys.path.insert(0, _p)
    import concourse  # noqa: F401

import concourse.bass as bass
import concourse.mybir as mybir
import concourse.tile as tile
from concourse import bacc
from concourse.bass_utils import run_bass_kernel_spmd
from concourse.masks import make_identity

F32 = mybir.dt.float32
F32R = mybir.dt.float32r
BF16 = mybir.dt.bfloat16

N_CORES = 8
P = 128
NTOK = 8192
DIN = 1024
DQK = 128
DV = 128
NLOC = NTOK // N_CORES  # 1024 rows per core
TQ = NLOC // P  # 8 q tiles per core
TD = DIN // P  # 8 d_in tiles
NKV = NTOK // P  # 64 kv tiles
SCH = 512  # kv chunk width (one fp32 psum bank)
NSCH = NTOK // SCH  # 16 chunks per q-tile
SCALE = 1.0 / float(np.sqrt(DQK))
MBIAS = -1.0  # extra downward bias on m_hat (A_max ~ e)
NEG_INF = -3.0e38


def build_nc():
    nc = bacc.Bacc(
        "TRN2",
        target_bir_lowering=False,
        debug=False,
        enable_asserts=False,
        num_devices=N_CORES,
    )

    x_d = nc.dram_tensor("x", [NLOC, DIN], F32, kind="ExternalInput").ap()
    wq_d = nc.dram_tensor("W_q", [DIN, DQK], F32, kind="ExternalInput").ap()
    wk_d = nc.dram_tensor("W_k", [DIN, DQK], F32, kind="ExternalInput").ap()
    wv_d = nc.dram_tensor("W_v", [DIN, DV], F32, kind="ExternalInput").ap()
    out_d = nc.dram_tensor("out", [NLOC, DV], F32, kind="ExternalOutput").ap()

    groups = [list(range(N_CORES))]

    with tile.TileContext(nc) as tc:
        with (
            tc.tile_pool(name="consts", bufs=1) as consts,
            tc.tile_pool(name="persist", bufs=1) as persist,
            tc.tile_pool(name="dram", bufs=1, space="DRAM") as dram,
        ):
            ident_f32 = consts.tile([P, P], F32)
            make_identity(nc, ident_f32)
            ident_bf = consts.tile([P, P], BF16)
            nc.vector.tensor_copy(out=ident_bf, in_=ident_f32)
            mbias_col = consts.tile([P, 1], F32)
            nc.vector.memset(mbias_col, MBIAS)

            # Persistent SBUF tensors.
            qT = persist.tile([P, NLOC], F32R)  # Q^T, pre-scaled, f32r
            kT_full = persist.tile([P, NTOK], F32R)
            vf = persist.tile([P, NKV, P], BF16)  # gathered V tiles
            kTl = persist.tile([P, NLOC], F32R)
            vl = persist.tile([P, TQ, P], BF16)
            mx_all = persist.tile([P, TQ, NSCH // 2], F32)  # per-pair maxes
            negm_col = persist.tile([P, TQ], F32)  # -(m_hat)+MBIAS
            den_parts = persist.tile([P, TQ, NSCH // 2], F32)

            # DRAM bounce buffers for the collectives. K and V gather
            # separately so AllGather(K) can fire as soon as the K
            # projection lands -- phase (a) only needs K.
            k_bounce = dram.tile([P, NLOC], F32R)
            k_gath = dram.tile([P * N_CORES, NLOC], F32R, addr_space="Shared")
            v_bounce = dram.tile([P, NLOC // 2], F32R)
            v_gath = dram.tile(
                [P * N_CORES, NLOC // 2], F32R, addr_space="Shared"
            )

            # ---------------- projections + collectives ----------------
            with (
                tc.tile_pool(name="proj_sb", bufs=1) as proj_sb,
                tc.tile_pool(name="ps_xt", bufs=2, space="PSUM") as ps_xt_pool,
                tc.tile_pool(name="ps_mm", bufs=2, space="PSUM") as ps_mm_pool,
                tc.tile_pool(name="ps_v", bufs=2, space="PSUM") as ps_v_pool,
            ):
                xa = proj_sb.tile([P, TQ, DIN], F32)
                xT_r = proj_sb.tile([P, TD, NLOC], F32R)
                xT_bf = proj_sb.tile([P, TD, NLOC], BF16)
                wq = proj_sb.tile([P, TD, DQK], F32)
                wk = proj_sb.tile([P, TD, DQK], F32)
                wv = proj_sb.tile([P, TD, DV], F32)
                wq_r = proj_sb.tile([P, TD, DQK], F32R)
                wk_r = proj_sb.tile([P, TD, DQK], F32R)
                wv_bf = proj_sb.tile([P, TD, DV], BF16)

                with nc.named_scope("load"):
                    # W_k first (it gates the K projection -> AllGather) and
                    # per-tile contiguous DMAs; x on the scalar queue so the
                    # ~630ns/DMA issue cost doesn't serialize the load phase.
                    for di in range(TD):
                        nc.sync.dma_start(
                            out=wk[:, di, :], in_=wk_d[di * P : (di + 1) * P, :]
                        )
                    for tj in range(TQ):
                        nc.scalar.dma_start(
                            out=xa[:, tj, :], in_=x_d[tj * P : (tj + 1) * P, :]
                        )
                    for di in range(TD):
                        nc.gpsimd.dma_start(
                            out=wv[:, di, :], in_=wv_d[di * P : (di + 1) * P, :]
                        )
                    for di in range(TD):
                        nc.sync.dma_start(
                            out=wq[:, di, :], in_=wq_d[di * P : (di + 1) * P, :]
                        )
                    nc.vector.tensor_copy(out=wk_r, in_=wk)
                    nc.vector.tensor_copy(out=wv_bf, in_=wv)
                    nc.vector.tensor_copy(out=wq_r, in_=wq)

                # x^T (PE transposes) and K^T projection per 512-token half,
                # so AllGather(K) can be issued as early as possible.
                for tg in range(2):
                    with nc.named_scope(f"xT_{tg}"):
                        for di in range(TD):
                            ps_xt = ps_xt_pool.tile([P, 4 * P], F32, tag="ps_xt")
                            for j in range(4):
                                tj = tg * 4 + j
                                nc.tensor.transpose(
                                    ps_xt[:, j * P : (j + 1) * P],
                                    xa[:, tj, di * P : (di + 1) * P],
                                    ident_f32,
                                )
                            sl = slice(tg * 4 * P, (tg + 1) * 4 * P)
                            nc.vector.tensor_copy(out=xT_r[:, di, sl], in_=ps_xt)
                            nc.vector.tensor_copy(
                                out=xT_bf[:, di, sl],
                                in_=xT_r[:, di, sl].bitcast(F32),
                            )
                    with nc.named_scope(f"kT_proj_{tg}"):
                        ps_k = ps_mm_pool.tile([P, 512], F32, tag="ps_mm")
                        for di in range(TD):
                            nc.tensor.matmul(
                                ps_k,
                                wk_r[:, di, :],
                                xT_r[:, di, tg * 512 : (tg + 1) * 512],
                                start=(di == 0),
                                stop=(di == TD - 1),
                            )
                        nc.vector.tensor_copy(
                            out=kTl[:, tg * 512 : (tg + 1) * 512], in_=ps_k
                        )
                        nc.sync.dma_start(
                            out=k_bounce[:, tg * 512 : (tg + 1) * 512],
                            in_=kTl[:, tg * 512 : (tg + 1) * 512],
                        )

                with nc.named_scope("ag_k"):
                    nc.gpsimd.collective_compute(
                        "AllGather",
                        mybir.AluOpType.bypass,
                        replica_groups=groups,
                        ins=[k_bounce.opt()],
                        outs=[k_gath.opt()],
                    )

                with nc.named_scope("v_proj"):
                    for tj in range(TQ):
                        ps_v = ps_v_pool.tile([P, DV], F32, tag="ps_v")
                        for di in range(TD):
                            nc.tensor.matmul(
                                ps_v,
                                xT_bf[:, di, tj * P : (tj + 1) * P],
                                wv_bf[:, di, :],
                                start=(di == 0),
                                stop=(di == TD - 1),
                            )
                        nc.vector.tensor_copy(out=vl[:, tj, :], in_=ps_v)
                    nc.sync.dma_start(out=v_bounce.bitcast(BF16), in_=vl)

                with nc.named_scope("ag_v"):
                    nc.gpsimd.collective_compute(
                        "AllGather",
                        mybir.AluOpType.bypass,
                        replica_groups=groups,
                        ins=[v_bounce.opt()],
                        outs=[v_gath.opt()],
                    )

                with nc.named_scope("q_proj"):
                    for h in range(NLOC // 512):
                        ps_q = ps_mm_pool.tile([P, 512], F32, tag="ps_mm")
                        for di in range(TD):
                            nc.tensor.matmul(
                                ps_q,
                                wq_r[:, di, :],
                                xT_r[:, di, h * 512 : (h + 1) * 512],
                                start=(di == 0),
                                stop=(di == TD - 1),
                            )
                        nc.vector.tensor_scalar_mul(
                            qT[:, h * 512 : (h + 1) * 512], ps_q, SCALE
                        )

                with nc.named_scope("gather_k"):
                    for c in range(N_CORES):
                        nc.sync.dma_start(
                            out=kT_full[:, c * NLOC : (c + 1) * NLOC],
                            in_=k_gath[c * P : (c + 1) * P, :],
                        )
                with nc.named_scope("gather_v"):
                    for c in range(N_CORES):
                        nc.scalar.dma_start(
                            out=vf[:, c * TQ : (c + 1) * TQ, :],
                            in_=v_gath[c * P : (c + 1) * P, :]
                            .bitcast(BF16)
                            .rearrange("p (t d) -> p t d", d=P),
                        )

            # ---------------- attention ----------------
            # s_sb lives in its own pool opened after the projection pool
            # closes, so the 64KB/partition S buffer reuses that space.
            with (
                tc.tile_pool(name="s_pool", bufs=1) as s_pool,
                tc.tile_pool(name="a_sb", bufs=4) as a_sb,
                tc.tile_pool(name="aT_sb", bufs=4) as aT_sb,
                tc.tile_pool(name="o_sb", bufs=2) as o_sb_pool,
                tc.tile_pool(name="stat_sb", bufs=2) as stat_sb,
                tc.tile_pool(name="ps_s", bufs=2, space="PSUM") as ps_s_pool,
                tc.tile_pool(name="ps_t", bufs=2, space="PSUM") as ps_t_pool,
                tc.tile_pool(name="ps_o", bufs=2, space="PSUM") as ps_o_pool,
            ):
                s_sb = s_pool.tile([P, 2, NSCH, SCH], F32)  # S rows, 2 tiles

                def emit_a(qt, c2):
                    """Two S chunk matmuls + ONE fused copy / row-max pass
                    over the 2-bank PSUM pair (halves DVE instruction count
                    and its per-op overhead)."""
                    ps_s = ps_s_pool.tile([P, 2, SCH], F32, tag="ps_s")
                    for h in range(2):
                        ch = 2 * c2 + h
                        nc.tensor.matmul(
                            ps_s[:, h, :],
                            qT[:, qt * P : (qt + 1) * P],
                            kT_full[:, ch * SCH : (ch + 1) * SCH],
                            start=True,
                            stop=True,
                        )
                    nc.vector.tensor_scalar(
                        out=s_sb[:, qt % 2, 2 * c2 : 2 * c2 + 2, :],
                        in0=ps_s,
                        scalar1=0.0,
                        scalar2=None,
                        op0=mybir.AluOpType.add,
                        op1=mybir.AluOpType.max,
                        accum_out=mx_all[:, qt, c2 : c2 + 1],
                    )

                def emit_negm(qt):
                    m1 = stat_sb.tile([P, 1], F32, tag="m1")
                    nc.vector.reduce_max(
                        m1, mx_all[:, qt, :], axis=mybir.AxisListType.X
                    )
                    nc.scalar.activation(
                        negm_col[:, qt : qt + 1],
                        m1,
                        mybir.ActivationFunctionType.Identity,
                        bias=mbias_col,
                        scale=-1.0,
                    )

                def emit_b(qt, c2, ps_o):
                    """ONE exp (+den partial) over a 1024-wide pair ->
                    A^T transposes -> ONE copy -> AV matmuls."""
                    a_ch = a_sb.tile([P, 2, SCH], BF16, tag="a")
                    nc.scalar.activation(
                        a_ch,
                        s_sb[:, qt % 2, 2 * c2 : 2 * c2 + 2, :],
                        mybir.ActivationFunctionType.Exp,
                        bias=negm_col[:, qt : qt + 1],
                        accum_out=den_parts[:, qt, c2 : c2 + 1],
                    )
                    ps_t = ps_t_pool.tile([P, 2, SCH], BF16, tag="ps_t")
                    af = a_ch.rearrange("p two s -> p (two s)")
                    tf = ps_t.rearrange("p two s -> p (two s)")
                    for j in range(2 * SCH // P):
                        nc.tensor.transpose(
                            tf[:, j * P : (j + 1) * P],
                            af[:, j * P : (j + 1) * P],
                            ident_bf,
                        )
                    aTb = aT_sb.tile([P, 2, SCH], BF16, tag="aT")
                    nc.vector.tensor_copy(out=aTb, in_=ps_t)
                    atf = aTb.rearrange("p two s -> p (two s)")
                    for j in range(2 * SCH // P):
                        kv = c2 * (2 * SCH // P) + j
                        nc.tensor.matmul(
                            ps_o,
                            atf[:, j * P : (j + 1) * P],
                            vf[:, kv, :],
                            start=(kv == 0),
                            stop=(kv == NKV - 1),
                        )

                def emit_out(qt, ps_o):
                    den = stat_sb.tile([P, 1], F32, tag="den")
                    nc.vector.reduce_sum(
                        den, den_parts[:, qt, :], axis=mybir.AxisListType.X
                    )
                    rden = stat_sb.tile([P, 1], F32, tag="rden")
                    nc.vector.reciprocal(rden, den)
                    o_nat = o_sb_pool.tile([P, DV], F32, tag="o")
                    nc.vector.tensor_scalar_mul(o_nat, ps_o, rden)
                    nc.sync.dma_start(
                        out=out_d[qt * P : (qt + 1) * P, :], in_=o_nat
                    )

                # software pipeline: phase (a) of tile qt overlaps (b) of qt-1
                ps_o_tiles = {}
                for qt in range(TQ + 1):
                    with nc.named_scope(f"attn_{qt}"):
                        if qt < TQ:
                            ps_o_new = ps_o_pool.tile([P, DV], F32, tag="ps_o")
                            ps_o_tiles[qt] = ps_o_new
                        for c2 in range(NSCH // 2):
                            if qt < TQ:
                                emit_a(qt, c2)
                            if qt >= 1:
                                emit_b(qt - 1, c2, ps_o_tiles[qt - 1])
                        if qt < TQ:
                            emit_negm(qt)
                        if qt >= 1:
                            emit_out(qt - 1, ps_o_tiles.pop(qt - 1))

    nc.compile()
    return nc


_NC_CACHE = None


def _get_nc():
    global _NC_CACHE
    if _NC_CACHE is None:
        _NC_CACHE = build_nc()
    return _NC_CACHE


def run(inputs, trace=False, **kw):
    """Run the SPMD kernel; returns BassKernelResults."""
    nc = _get_nc()
    x = np.asarray(inputs["x"], dtype=np.float32)
    wq = np.asarray(inputs["W_q"], dtype=np.float32)
    wk = np.asarray(inputs["W_k"], dtype=np.float32)
    wv = np.asarray(inputs["W_v"], dtype=np.float32)
    in_maps = [
        {
            "x": np.ascontiguousarray(x[c * NLOC : (c + 1) * NLOC]),
            "W_q": wq,
            "W_k": wk,
            "W_v": wv,
        }
        for c in range(N_CORES)
    ]
    return run_bass_kernel_spmd(
        nc, in_maps, core_ids=list(range(N_CORES)), trace=trace, **kw
    )


def kernel(**inputs):
    res = run(inputs, trace=False)
    return np.concatenate([res.results[c]["out"] for c in range(N_CORES)], axis=0)


# revision 35
# speedup vs baseline: 1.0872x; 1.0872x over previous
"""Distributed attention kernel for 8 TRN2 NeuronCores (natural-layout pipeline).

Problem: x[8192,1024] @ {W_q,W_k,W_v}[1024,128] -> softmax(QK^T/sqrt(128)) @ V.

Sharding: x row-sharded (1024 rows/core), weights replicated. Each core
computes K^T_loc/V_loc from its shard, AllGathers K^T (f32r) and V (bf16),
then attends its own 1024 Q rows against the full K/V.

Per-core attention (natural layout: q on partitions), per 128-row q-tile,
software-pipelined across tiles so phase (a) of tile t+1 overlaps (b) of t:
  (a) S pass: for each 512-kv chunk, S = Q_tile.T-stationary @ K^T-moving
      (f32r, one matmul); ONE fused DVE op (tensor_tensor_reduce) copies the
      PSUM chunk to SBUF *and* folds its row-max into a running max. S is
      computed exactly once (no separate stats matmul pass).
  (b) exp/AV pass: negm = -(m_hat)-1; for each chunk:
        A = exp(S_sbuf + negm)  [ACT: per-partition bias = per-q shift; the
            same instruction's accum_out emits the per-q denominator partial
            -- den costs no matmuls at all]
        A^T blocks via PE transpose (bf16) -> PSUM -> SBUF copy (split
            between DVE and ACT to balance engine load)
        O[q,dv] += A^T_block-stationary @ V_tile-moving  [bf16]
      Then den = sum of partials, O *= 1/den (DVE), direct DMA out (output
      is already in natural row layout -- no final transposes).

Numerics: logits have std ~1024 (randn inputs); softmax is near-one-hot so
the Q/K/S path needs |logit err| << 1: f32r (11-bit mantissa) gives ~0.15.
m_hat is the exact f32 row max of the f32r logits; the -1 bias is cosmetic.
"""

import os
import sys

import numpy as np

os.environ.setdefault("MYCRO_LOCAL_CACHE", "1")

try:
    import concourse  # noqa: F401
except ImportError:  # pragma: no cover - path fallback for fresh dirs
    for _p in ("/opt/trn_rl_repo", "/root/.axon_site/_ro/trn_rl_repo"):
        if os.path.isdir(_p):
            sys.path.insert(0, _p)
    import concourse  # noqa: F401

import concourse.bass as bass
import concourse.mybir as mybir
import concourse.tile as tile
from concourse import bacc
from concourse.bass_utils import run_bass_kernel_spmd
from concourse.masks import make_identity

F32 = mybir.dt.float32
F32R = mybir.dt.float32r
BF16 = mybir.dt.bfloat16

N_CORES = 8
P = 128
NTOK = 8192
DIN = 1024
DQK = 128
DV = 128
NLOC = NTOK // N_CORES  # 1024 rows per core
TQ = NLOC // P  # 8 q tiles per core
TD = DIN // P  # 8 d_in tiles
NKV = NTOK // P  # 64 kv tiles
SCH = 512  # kv chunk width (one fp32 psum bank)
NSCH = NTOK // SCH  # 16 chunks per q-tile
SCALE = 1.0 / float(np.sqrt(DQK))
MBIAS = -1.0  # extra downward bias on m_hat (A_max ~ e)
NEG_INF = -3.0e38


def build_nc():
    nc = bacc.Bacc(
        "TRN2",
        target_bir_lowering=False,
        debug=False,
        enable_asserts=False,
        num_devices=N_CORES,
    )

    x_d = nc.dram_tensor("x", [NLOC, DIN], F32, kind="ExternalInput").ap()
    wq_d = nc.dram_tensor("W_q", [DIN, DQK], F32, kind="ExternalInput").ap()
    wk_d = nc.dram_tensor("W_k", [DIN, DQK], F32, kind="ExternalInput").ap()
    wv_d = nc.dram_tensor("W_v", [DIN, DV], F32, kind="ExternalInput").ap()
    out_d = nc.dram_tensor("out", [NLOC, DV], F32, kind="ExternalOutput").ap()

    groups = [list(range(N_CORES))]

    with tile.TileContext(nc) as tc:
        with (
            tc.tile_pool(name="consts", bufs=1) as consts,
            tc.tile_pool(name="persist", bufs=1) as persist,
            tc.tile_pool(name="dram", bufs=1, space="DRAM") as dram,
        ):
            ident_f32 = consts.tile([P, P], F32)
            make_identity(nc, ident_f32)
            ident_bf = consts.tile([P, P], BF16)
            nc.vector.tensor_copy(out=ident_bf, in_=ident_f32)
            mbias_col = consts.tile([P, 1], F32)
            nc.vector.memset(mbias_col, MBIAS)

            # Persistent SBUF tensors.
            qT = persist.tile([P, NLOC], F32R)  # Q^T, pre-scaled, f32r
            kT_full = persist.tile([P, NTOK], F32R)
            vf = persist.tile([P, NKV, P], BF16)  # gathered V tiles
            kTl = persist.tile([P, NLOC], F32R)
            vl = persist.tile([P, TQ, P], BF16)
            mx_all = persist.tile([P, TQ, NSCH], F32)  # per-chunk row maxes
            negm_col = persist.tile([P, TQ], F32)  # -(m_hat)+MBIAS
            den_parts = persist.tile([P, TQ, NSCH], F32)

            # DRAM bounce buffers for the collectives. K and V gather
            # separately so AllGather(K) can fire as soon as the K
            # projection lands -- phase (a) only needs K.
            k_bounce = dram.tile([P, NLOC], F32R)
            k_gath = dram.tile([P * N_CORES, NLOC], F32R, addr_space="Shared")
            v_bounce = dram.tile([P, NLOC // 2], F32R)
            v_gath = dram.tile(
                [P * N_CORES, NLOC // 2], F32R, addr_space="Shared"
            )

            # ---------------- projections + collectives ----------------
            with (
                tc.tile_pool(name="proj_sb", bufs=1) as proj_sb,
                tc.tile_pool(name="ps_xt", bufs=2, space="PSUM") as ps_xt_pool,
                tc.tile_pool(name="ps_mm", bufs=2, space="PSUM") as ps_mm_pool,
                tc.tile_pool(name="ps_v", bufs=2, space="PSUM") as ps_v_pool,
            ):
                xa = proj_sb.tile([P, TQ, DIN], F32)
                xT_r = proj_sb.tile([P, TD, NLOC], F32R)
                xT_bf = proj_sb.tile([P, TD, NLOC], BF16)
                wq = proj_sb.tile([P, TD, DQK], F32)
                wk = proj_sb.tile([P, TD, DQK], F32)
                wv = proj_sb.tile([P, TD, DV], F32)
                wq_r = proj_sb.tile([P, TD, DQK], F32R)
                wk_r = proj_sb.tile([P, TD, DQK], F32R)
                wv_bf = proj_sb.tile([P, TD, DV], BF16)

                with nc.named_scope("load"):
                    # W_k first (it gates the K projection -> AllGather) and
                    # per-tile contiguous DMAs; x on the scalar queue so the
                    # ~630ns/DMA issue cost doesn't serialize the load phase.
                    for di in range(TD):
                        nc.sync.dma_start(
                            out=wk[:, di, :], in_=wk_d[di * P : (di + 1) * P, :]
                        )
                    for tj in range(TQ):
                        nc.scalar.dma_start(
                            out=xa[:, tj, :], in_=x_d[tj * P : (tj + 1) * P, :]
                        )
                    for di in range(TD):
                        nc.gpsimd.dma_start(
                            out=wv[:, di, :], in_=wv_d[di * P : (di + 1) * P, :]
                        )
                    for di in range(TD):
                        nc.sync.dma_start(
                            out=wq[:, di, :], in_=wq_d[di * P : (di + 1) * P, :]
                        )
                    nc.vector.tensor_copy(out=wk_r, in_=wk)
                    nc.vector.tensor_copy(out=wv_bf, in_=wv)
                    nc.vector.tensor_copy(out=wq_r, in_=wq)

                # x^T (PE transposes) and K^T projection per 512-token half,
                # so AllGather(K) can be issued as early as possible.
                for tg in range(2):
                    with nc.named_scope(f"xT_{tg}"):
                        for di in range(TD):
                            ps_xt = ps_xt_pool.tile([P, 4 * P], F32, tag="ps_xt")
                            for j in range(4):
                                tj = tg * 4 + j
                                nc.tensor.transpose(
                                    ps_xt[:, j * P : (j + 1) * P],
                                    xa[:, tj, di * P : (di + 1) * P],
                                    ident_f32,
                                )
                            sl = slice(tg * 4 * P, (tg + 1) * 4 * P)
                            nc.vector.tensor_copy(out=xT_r[:, di, sl], in_=ps_xt)
                            nc.vector.tensor_copy(
                                out=xT_bf[:, di, sl],
                                in_=xT_r[:, di, sl].bitcast(F32),
                            )
                    with nc.named_scope(f"kT_proj_{tg}"):
                        ps_k = ps_mm_pool.tile([P, 512], F32, tag="ps_mm")
                        for di in range(TD):
                            nc.tensor.matmul(
                                ps_k,
                                wk_r[:, di, :],
                                xT_r[:, di, tg * 512 : (tg + 1) * 512],
                                start=(di == 0),
                                stop=(di == TD - 1),
                            )
                        nc.vector.tensor_copy(
                            out=kTl[:, tg * 512 : (tg + 1) * 512], in_=ps_k
                        )
                        nc.sync.dma_start(
                            out=k_bounce[:, tg * 512 : (tg + 1) * 512],
                            in_=kTl[:, tg * 512 : (tg + 1) * 512],
                        )

                with nc.named_scope("ag_k"):
                    nc.gpsimd.collective_compute(
                        "AllGather",
                        mybir.AluOpType.bypass,
                        replica_groups=groups,
                        ins=[k_bounce.opt()],
                        outs=[k_gath.opt()],
                    )

                with nc.named_scope("v_proj"):
                    for tj in range(TQ):
                        ps_v = ps_v_pool.tile([P, DV], F32, tag="ps_v")
                        for di in range(TD):
                            nc.tensor.matmul(
                                ps_v,
                                xT_bf[:, di, tj * P : (tj + 1) * P],
                                wv_bf[:, di, :],
                                start=(di == 0),
                                stop=(di == TD - 1),
                            )
                        nc.vector.tensor_copy(out=vl[:, tj, :], in_=ps_v)
                    nc.sync.dma_start(out=v_bounce.bitcast(BF16), in_=vl)

                with nc.named_scope("ag_v"):
                    nc.gpsimd.collective_compute(
                        "AllGather",
                        mybir.AluOpType.bypass,
                        replica_groups=groups,
                        ins=[v_bounce.opt()],
                        outs=[v_gath.opt()],
                    )

                with nc.named_scope("q_proj"):
                    for h in range(NLOC // 512):
                        ps_q = ps_mm_pool.tile([P, 512], F32, tag="ps_mm")
                        for di in range(TD):
                            nc.tensor.matmul(
                                ps_q,
                                wq_r[:, di, :],
                                xT_r[:, di, h * 512 : (h + 1) * 512],
                                start=(di == 0),
                                stop=(di == TD - 1),
                            )
                        nc.vector.tensor_scalar_mul(
                            qT[:, h * 512 : (h + 1) * 512], ps_q, SCALE
                        )

                with nc.named_scope("gather_k"):
                    for c in range(N_CORES):
                        nc.sync.dma_start(
                            out=kT_full[:, c * NLOC : (c + 1) * NLOC],
                            in_=k_gath[c * P : (c + 1) * P, :],
                        )
                with nc.named_scope("gather_v"):
                    for c in range(N_CORES):
                        nc.scalar.dma_start(
                            out=vf[:, c * TQ : (c + 1) * TQ, :],
                            in_=v_gath[c * P : (c + 1) * P, :]
                            .bitcast(BF16)
                            .rearrange("p (t d) -> p t d", d=P),
                        )

            # ---------------- attention ----------------
            # s_sb lives in its own pool opened after the projection pool
            # closes, so the 64KB/partition S buffer reuses that space.
            with (
                tc.tile_pool(name="s_pool", bufs=1) as s_pool,
                tc.tile_pool(name="a_sb", bufs=4) as a_sb,
                tc.tile_pool(name="aT_sb", bufs=6) as aT_sb,
                tc.tile_pool(name="o_sb", bufs=2) as o_sb_pool,
                tc.tile_pool(name="stat_sb", bufs=2) as stat_sb,
                tc.tile_pool(name="ps_s", bufs=2, space="PSUM") as ps_s_pool,
                tc.tile_pool(name="ps_t", bufs=3, space="PSUM") as ps_t_pool,
                tc.tile_pool(name="ps_o", bufs=2, space="PSUM") as ps_o_pool,
            ):
                s_sb = s_pool.tile([P, 2, NSCH, SCH], F32)  # S rows, 2 tiles

                def emit_a(qt, ch):
                    """S chunk matmul + fused SBUF copy / per-chunk row-max."""
                    ps_s = ps_s_pool.tile([P, SCH], F32, tag="ps_s")
                    nc.tensor.matmul(
                        ps_s,
                        qT[:, qt * P : (qt + 1) * P],
                        kT_full[:, ch * SCH : (ch + 1) * SCH],
                        start=True,
                        stop=True,
                    )
                    # one DVE pass: copy PSUM chunk to SBUF and emit its
                    # per-row max (tensor_scalar's built-in accumulator)
                    nc.vector.tensor_scalar(
                        out=s_sb[:, qt % 2, ch, :],
                        in0=ps_s,
                        scalar1=0.0,
                        scalar2=None,
                        op0=mybir.AluOpType.add,
                        op1=mybir.AluOpType.max,
                        accum_out=mx_all[:, qt, ch : ch + 1],
                    )

                def emit_negm(qt):
                    m1 = stat_sb.tile([P, 1], F32, tag="m1")
                    nc.vector.reduce_max(
                        m1, mx_all[:, qt, :], axis=mybir.AxisListType.X
                    )
                    nc.scalar.activation(
                        negm_col[:, qt : qt + 1],
                        m1,
                        mybir.ActivationFunctionType.Identity,
                        bias=mbias_col,
                        scale=-1.0,
                    )

                def emit_b(qt, ch, ps_o):
                    """exp (+den partial) -> A^T transpose -> AV matmuls."""
                    a_ch = a_sb.tile([P, SCH], BF16, tag="a")
                    nc.scalar.activation(
                        a_ch,
                        s_sb[:, qt % 2, ch, :],
                        mybir.ActivationFunctionType.Exp,
                        bias=negm_col[:, qt : qt + 1],
                        accum_out=den_parts[:, qt, ch : ch + 1],
                    )
                    ps_t = ps_t_pool.tile([P, SCH], BF16, tag="ps_t")
                    for j in range(SCH // P):
                        nc.tensor.transpose(
                            ps_t[:, j * P : (j + 1) * P],
                            a_ch[:, j * P : (j + 1) * P],
                            ident_bf,
                        )
                    aTb = aT_sb.tile([P, SCH], BF16, tag="aT")
                    # A^T copies mostly on the DVE; every 4th on the ACT to
                    # shave the DVE's lead as the binding engine
                    if ch % 4 == 3:
                        nc.scalar.copy(out=aTb, in_=ps_t)
                    else:
                        nc.vector.tensor_copy(out=aTb, in_=ps_t)
                    for j in range(SCH // P):
                        kv = ch * (SCH // P) + j
                        nc.tensor.matmul(
                            ps_o,
                            aTb[:, j * P : (j + 1) * P],
                            vf[:, kv, :],
                            start=(kv == 0),
                            stop=(kv == NKV - 1),
                        )

                def emit_out(qt, ps_o):
                    den = stat_sb.tile([P, 1], F32, tag="den")
                    nc.vector.reduce_sum(
                        den, den_parts[:, qt, :], axis=mybir.AxisListType.X
                    )
                    rden = stat_sb.tile([P, 1], F32, tag="rden")
                    nc.vector.reciprocal(rden, den)
                    o_nat = o_sb_pool.tile([P, DV], F32, tag="o")
                    nc.vector.tensor_scalar_mul(o_nat, ps_o, rden)
                    nc.sync.dma_start(
                        out=out_d[qt * P : (qt + 1) * P, :], in_=o_nat
                    )

                # software pipeline: phase (a) of tile qt overlaps (b) of qt-1
                ps_o_tiles = {}
                for qt in range(TQ + 1):
                    with nc.named_scope(f"attn_{qt}"):
                        if qt < TQ:
                            ps_o_new = ps_o_pool.tile([P, DV], F32, tag="ps_o")
                            ps_o_tiles[qt] = ps_o_new
                        for ch in range(NSCH):
                            if qt < TQ:
                                emit_a(qt, ch)
                            if qt >= 1:
                                emit_b(qt - 1, ch, ps_o_tiles[qt - 1])
                        if qt < TQ:
                            emit_negm(qt)
                        if qt >= 1:
                            emit_out(qt - 1, ps_o_tiles.pop(qt - 1))

    nc.compile()
    return nc


_NC_CACHE = None


def _get_nc():
    global _NC_CACHE
    if _NC_CACHE is None:
        _NC_CACHE = build_nc()
    return _NC_CACHE


def run(inputs, trace=False, **kw):
    """Run the SPMD kernel; returns BassKernelResults."""
    nc = _get_nc()
    x = np.asarray(inputs["x"], dtype=np.float32)
    wq = np.asarray(inputs["W_q"], dtype=np.float32)
    wk = np.asarray(inputs["W_k"], dtype=np.float32)
    wv = np.asarray(inputs["W_v"], dtype=np.float32)
    in_maps = [
        {
            "x": np.ascontiguousarray(x[c * NLOC : (c + 1) * NLOC]),
            "W_q": wq,
            "W_k": wk,
            "W_v": wv,
        }
        for c in range(N_CORES)
    ]
    return run_bass_kernel_spmd(
        nc, in_maps, core_ids=list(range(N_CORES)), trace=trace, **kw
    )


def kernel(**inputs):
    res = run(inputs, trace=False)
    return np.concatenate([res.results[c]["out"] for c in range(N_CORES)], axis=0)
